# revision 2
# baseline (speedup 1.0000x reference)
"""Trainium2 Bass kernel for CustomSTFT — rank-2 decomposition version.

The roundtrip matrix M = w_bwd_r.T@w_fwd_r - w_bwd_i.T@w_fwd_i collapses to
    M = 0.5*diag(win^2) + (1/800)*(we we^T + wo wo^T)
(diagonal + rank-2, parity-split), and hann^2 OLA at hop 200 is exactly 1.5, so

    y[200t+r] = 0.75*xp[200t+r] + (1/800) * sum_{d=0..3} win[200d+r]*a_{p(r)}[t-d]
    a_q[f]    = sum_{j%2==q} win[j]*xp[200f+j]

plus envelope/tap fixes at the two edge output chunks. Per core (2 samples)
the PE streams ~14k columns instead of the baseline's 67k.

Layouts: output chunks are blocked k = 4t+b (t in [0,3), b in [0,4)); the
G / Ash intermediates live on sparse 128-partition tiles with 8-row slabs at
32-partition bases {0,32,64,96} (the only legal PE tile positions):

  stage A : G_t[32b+2d'+q, j] = quarter-window partial dots (24 mm / sample)
  AB      : As_t = a_p values via 7 block-diag shift-selector matmuls
  stage B : y2[r', o] = W2 @ As slabs; tiny identity matmuls inject the edge
            deficit chunks; combine y = y2_psum + 0.75*xp on DVE/Pool/Act.

Host marshals inputs to a zero-padded chunked fp16 layout xs[s', ss, h, j]
(j = chunk index + 3; deficit chunks at 1222/1223); output fp16 -> f32.
"""

import numpy as np

N_CORES = 8
B, T = 16, 240000
SPC = B // N_CORES
C, H = 200, 100
NO = T // C                 # 1200 output chunks / sample
NK, L = 12, 101             # k-blocks of output chunks, 12*101 >= 1200
NT = 4                      # k-tiles (3 k-blocks each at bases 0/32/64)
NCH = 1224                  # host chunk array: 3 zero | 1204 real | 15 zero | 2 special
FG = 111                    # G free width per k-block (chunks 101k-3 .. 101k+107)
GP = 404                    # output-chunk group width (4 k-blocks)
N_WARM = 32

# wt column layout
W1_OFF = 0                  # 2 x [100, 32]: cols 8..32 zero so stage A zero-fills gap rows
W2_OFF = 64                 # 10 x [96, 100] full-contraction W2, slab at 32*(k%3)
I_OFF = 1064                # [100, 100] identity
S_OFF = 1164                # 7 x [96, 96] block-diag shift selectors
WT_COLS = 1164 + 7 * 96     # 1836


def _w2_idx(v, blk, b):
    # v0 (all taps): per slab position b; v1 = noD3 (k=0, b=0); v2 = noD0 (k=11, b=2)
    if v == 0:
        return b * 2 + blk
    return (4 + 2 * v) + blk

_cache = {}


def _host_weights():
    if "wt" in _cache:
        return _cache["wt"]
    win = 0.5 * (1.0 - np.cos(2.0 * np.pi * np.arange(800) / 800))
    wt = np.zeros((100, WT_COLS), dtype=np.float32)
    par = np.arange(100) % 2
    # W1[s', 32h + 2d' + q] = win[200d' + 100h + s'] / 0.75 * [s'%2 == q]
    for h in range(2):
        for dp in range(4):
            for q in range(2):
                wt[0:100, W1_OFF + 32 * h + 2 * dp + q] = (
                    win[200 * dp + 100 * h:200 * dp + 100 * h + 100] / 0.75 * (par == q))
    # W2 full-96-contraction variants: nonzero only in slab rows 32b + 2d + p
    for v in range(3):
        for blk in range(2):
            bbs = range(3) if v == 0 else ([0] if v == 1 else [2])
            for bb in bbs:
                col0 = W2_OFF + _w2_idx(v, blk, bb) * 100
                for d in range(4):
                    if (v == 1 and d == 3) or (v == 2 and d == 0):
                        continue
                    for p in range(2):
                        wt[32 * bb + 2 * d + p, col0:col0 + 100] = (
                            win[200 * d + 100 * blk:200 * d + 100 * blk + 100]
                            / 800.0 * (par == p))
    # identity
    wt[0:100, I_OFF:I_OFF + 100] = np.eye(100)
    # shift selectors S_e: [32b + 2d' + q, 32b + 2(d'+e) + q] = 1
    for ep in range(7):
        e = ep - 3
        col0 = S_OFF + 96 * ep
        for bb in range(3):
            for dp in range(4):
                d = dp + e
                if 0 <= d <= 3:
                    for q in range(2):
                        wt[32 * bb + 2 * dp + q, col0 + 32 * bb + 2 * d + q] = 1.0
    wt = wt.astype(np.float16)
    _cache["wt"] = wt
    return wt


def _build_nc():
    if "nc" in _cache:
        return _cache["nc"]
    import concourse.mybir as mybir
    import concourse.tile as tile
    from concourse import bacc

    f32 = mybir.dt.float32
    f16 = mybir.dt.float16

    nc = bacc.Bacc("TRN2", target_bir_lowering=False, debug=False,
                   num_devices=N_CORES)
    xs_d = nc.dram_tensor("xs", [100, SPC, 2, NCH], f16, kind="ExternalInput").ap()
    wt_d = nc.dram_tensor("wt", [100, WT_COLS], f16, kind="ExternalInput").ap()
    out_d = nc.dram_tensor("out", [100, SPC, 2, NO], f16, kind="ExternalOutput").ap()

    with tile.TileContext(nc) as tc:
        with (
            tc.tile_pool(name="weights", bufs=1) as wpool,
            tc.tile_pool(name="data", bufs=1) as dpool,
            tc.tile_pool(name="pg", bufs=2, space="PSUM") as pg,
            tc.tile_pool(name="pa", bufs=2, space="PSUM") as pa,
            tc.tile_pool(name="py", bufs=3, space="PSUM") as py,
        ):
            wts = wpool.tile([100, WT_COLS], f16)
            warm = wpool.tile([100, 100], f16)
            xps = dpool.tile([100, SPC, 2, NCH], f16)
            gsb = dpool.tile([96, SPC, NT, FG], f16)
            ashsb = dpool.tile([96, SPC, NT, L], f16)
            ysb = dpool.tile([100, SPC, 2, NO], f16)

            nc.gpsimd.memset(warm[:], 0.0)

            # --- input DMA: sync lane = sample pieces, scalar lane = weights
            nc.sync.dma_start(xps[:, 0, :, 0:640], xs_d[:, 0, :, 0:640])
            nc.sync.dma_start(xps[:, 0, :, 640:NCH], xs_d[:, 0, :, 640:NCH])
            nc.sync.dma_start(xps[:, 1, :, 0:640], xs_d[:, 1, :, 0:640])
            nc.sync.dma_start(xps[:, 1, :, 640:NCH], xs_d[:, 1, :, 640:NCH])
            nc.scalar.dma_start(wts[:, 0:64], wt_d[:, 0:64])
            nc.scalar.dma_start(wts[:, S_OFF:WT_COLS], wt_d[:, S_OFF:WT_COLS])
            nc.scalar.dma_start(wts[:, 64:S_OFF], wt_d[:, 64:S_OFF])

            # --- PE warmup while DMA is in flight (HAM clock ramp) ---
            wps = py.tile([100, GP], f32, tag="yg")
            for _ in range(N_WARM):
                nc.tensor.matmul(wps[:, 0:100], warm[:], warm[:],
                                 start=True, stop=True)

            # --- stage A: G_t sparse tiles, k = 3t+b at base 32b ---
            for ss in range(SPC):
                for t in range(NT):
                    g = pg.tile([96, FG], f32, tag="g")
                    for bb in range(3):
                        k = 3 * t + bb
                        for h in range(2):
                            nc.tensor.matmul(
                                g[32 * bb:32 * bb + 32, :],
                                wts[0:100, 32 * h:32 * h + 32],
                                xps[:, ss, h, 101 * k:101 * k + FG],
                                start=(h == 0), stop=(h == 1),
                                tile_position=(0, 32 * bb))
                    nc.scalar.copy(gsb[:, ss, t, :], g[:, :])

            # --- AB: As_t[32b+2d+p, t'] = a_p[101(3t+b) + t' + 2 - d] ---
            for ss in range(SPC):
                for t in range(NT):
                    a = pa.tile([96, L], f32, tag="a")
                    for ep in range(7):
                        nc.tensor.matmul(
                            a[:, :],
                            wts[0:96, S_OFF + 96 * ep:S_OFF + 96 * ep + 96],
                            gsb[:, ss, t, 8 - ep:8 - ep + L],
                            start=(ep == 0), stop=(ep == 6))
                    nc.scalar.copy(ashsb[:, ss, t, :], a[:, :])

            ident = wts[:, I_OFF:I_OFF + 100]

            def kmm(y, ss, g, i, v=0, blk=0, cols=None, acols=None, **fl):
                """stage-B matmul for k-block k = 4g+i into group psum y.
                Full-96 contraction: W2 matrix is zero outside slab 32*(k%3)."""
                k = 4 * g + i
                t, bb = divmod(k, 3)
                c = 101 * i
                c0, c1 = cols if cols is not None else (0, 101)
                a0, a1 = acols if acols is not None else (0, 101)
                w2c = W2_OFF + _w2_idx(v, blk, bb) * 100
                nc.tensor.matmul(y[:, c + c0:c + c1],
                                 wts[0:96, w2c:w2c + 100],
                                 ashsb[0:96, ss, t, a0:a1], **fl)

            def stage_b(ss, blk):
                # g0: o [0,404): k 0..3; col 0 special (noD3 + deficit chunk 1222)
                y0 = py.tile([100, GP], f32, tag="yg")
                nc.tensor.matmul(y0[:, 0:1], ident, xps[:, ss, blk, 1222:1223],
                                 start=True, stop=False, skip_group_check=True)
                kmm(y0, ss, 0, 0, v=1, blk=blk, cols=(0, 1), acols=(0, 1),
                    start=False, stop=True, skip_group_check=True)
                kmm(y0, ss, 0, 0, blk=blk, cols=(1, 101), acols=(1, 101),
                    start=True, stop=True)
                for i in range(1, 4):
                    kmm(y0, ss, 0, i, blk=blk, start=True, stop=True)
                nc.vector.tensor_add(ysb[:, ss, blk, 0:GP], y0[:, :],
                                     xps[:, ss, blk, 5:5 + GP])
                # g1: o [404,808): k 4..7; ss1 pure-copy, ss0 DVE add.
                # Pure groups accumulate 0.75*xp via per-region identity
                # matmuls (a spanning accumulate across region starts breaks
                # PSUM accumulation-group tracking).
                y1 = py.tile([100, GP], f32, tag="yg")
                pure1 = ss == 1
                for i in range(4):
                    kmm(y1, ss, 1, i, blk=blk, start=True, stop=not pure1,
                        skip_group_check=pure1)
                    if pure1:
                        c = 101 * i
                        nc.tensor.matmul(y1[:, c:c + 101], ident,
                                         xps[:, ss, blk, 5 + GP + c:5 + GP + c + 101],
                                         start=False, stop=True,
                                         skip_group_check=True)
                if pure1:
                    nc.scalar.copy(ysb[:, ss, blk, GP:2 * GP], y1[:, :])
                else:
                    nc.vector.tensor_add(ysb[:, ss, blk, GP:2 * GP], y1[:, :],
                                         xps[:, ss, blk, 5 + GP:5 + 2 * GP])
                # g2: o [808,1200): k 8..11; col 391 special (noD0 + deficit 1223);
                # pure-copy group with per-region identity matmuls.
                y2 = py.tile([100, GP], f32, tag="yg")
                x2 = 5 + 2 * GP
                for i in range(3):
                    c = 101 * i
                    kmm(y2, ss, 2, i, blk=blk, start=True, stop=False,
                        skip_group_check=True)
                    nc.tensor.matmul(y2[:, c:c + 101], ident,
                                     xps[:, ss, blk, x2 + c:x2 + c + 101],
                                     start=False, stop=True, skip_group_check=True)
                kmm(y2, ss, 2, 3, blk=blk, cols=(0, 88), acols=(0, 88),
                    start=True, stop=False, skip_group_check=True)
                nc.tensor.matmul(y2[:, 303:391], ident,
                                 xps[:, ss, blk, x2 + 303:x2 + 391],
                                 start=False, stop=True, skip_group_check=True)
                kmm(y2, ss, 2, 3, v=2, blk=blk, cols=(88, 89), acols=(88, 89),
                    start=True, stop=False, skip_group_check=True)
                nc.tensor.matmul(y2[:, 391:392], ident, xps[:, ss, blk, 1223:1224],
                                 start=False, stop=False, skip_group_check=True)
                nc.tensor.matmul(y2[:, 391:392], ident,
                                 xps[:, ss, blk, x2 + 391:x2 + 392],
                                 start=False, stop=True, skip_group_check=True)
                nc.scalar.copy(ysb[:, ss, blk, 2 * GP:NO], y2[:, 0:392])

            stage_b(0, 0)
            stage_b(0, 1)
            stage_b(1, 0)
            stage_b(1, 1)

            nc.sync.dma_start(out_d[:, 0, 0, :], ysb[:, 0, 0, :])
            nc.sync.dma_start(out_d[:, 0, 1, :], ysb[:, 0, 1, :])
            nc.sync.dma_start(out_d[:, 1, 0, :], ysb[:, 1, 0, :])
            nc.sync.dma_start(out_d[:, 1, 1, 0:2 * GP], ysb[:, 1, 1, 0:2 * GP])
            nc.sync.dma_start(out_d[:, 1, 1, 2 * GP:NO], ysb[:, 1, 1, 2 * GP:NO])

    nc.compile()
    _cache["nc"] = nc
    return nc


last_results = None


def kernel(x, w_fwd_r=None, w_fwd_i=None, w_bwd_r=None, w_bwd_i=None):
    global last_results
    from concourse.bass_utils import run_bass_kernel_spmd

    x = np.asarray(x, dtype=np.float32)
    assert x.shape == (B, T), x.shape
    WT = _host_weights()
    nc = _build_nc()

    win = 0.5 * (1.0 - np.cos(2.0 * np.pi * np.arange(800) / 800))
    xp = np.pad(x, ((0, 0), (400, 400)), mode="edge")          # [B, 240800]
    xs32 = (0.75 * xp).reshape(B, 1204, 2, 100)                # (b, c, h, s')
    spcA = (-0.5 * win[600:800] ** 2)[None, :] * xp[:, 400:600]     # (b, s)
    spcB = (-0.5 * win[0:200] ** 2)[None, :] * xp[:, 240200:240400]

    in_maps = []
    for core in range(N_CORES):
        arr = np.zeros((100, SPC, 2, NCH), dtype=np.float16)
        for ss in range(SPC):
            b = core * SPC + ss
            arr[:, ss, :, 3:1207] = xs32[b].transpose(2, 1, 0).astype(np.float16)
            arr[:, ss, :, 1222] = spcA[b].reshape(2, 100).T.astype(np.float16)
            arr[:, ss, :, 1223] = spcB[b].reshape(2, 100).T.astype(np.float16)
        in_maps.append({"xs": arr, "wt": WT})

    res = run_bass_kernel_spmd(nc, in_maps, core_ids=list(range(N_CORES)))
    last_results = res

    y = np.empty((B, T), dtype=np.float32)
    for core in range(N_CORES):
        od = res.results[core]["out"]                          # [100, SPC, 2, NO]
        for ss in range(SPC):
            y[core * SPC + ss] = od[:, ss].transpose(2, 1, 0).reshape(T)
    return y
